# revision 24
# baseline (speedup 1.0000x reference)
"""Trainium2 Bass kernel for a ViT attention block (LN->MHA+relpos->LN->MLP).

Contract: kernel(**inputs) takes the FULL unsharded inputs, shards batch
across 8 NeuronCores (4 items per core), runs one SPMD Bass program, and
gathers the full [32, 577, 768] fp32 output.

v3 design notes (vs. baseline):
- S matmuls run as row-group pairs: head 2j uses PE rows 0-63, head 2j+1
  rows 64-127, concurrently (tile_position auto-derived from the base
  partitions). Halves the S matmul stream time.
- Rel-pos bias still accumulated by an identity-weight matmul (K=128), but
  per head pair it serializes after the concurrent S pair.
- Softmax denominator rides the PV matmul as a ones-column; den/o drains,
  reciprocal and the DRAM-round-trip broadcast follow the baseline scheme
  (DVE drains for o, ACT drains for den).
- LN1 stats run in a prologue over all items (batched sqrt, one activation
  table set switch); the normalize runs as replay steps just-in-time.
  LN2 sqrt is batched per item. Exp is the only hot ACT table during
  attention.
- Normalized o is written back into the kT slab (dead after S) for proj.
- x2 (residual stream after attention) is stored bf16.
- MLP weights load inside the MLP scope; activations 512-token blocks.
"""

import sys

if '/opt/trn_rl_repo' not in sys.path:
    sys.path.insert(0, '/opt/trn_rl_repo')

from contextlib import ExitStack

import numpy as np
import ml_dtypes

import concourse.bass as bass  # noqa: F401
import concourse.tile as tile
import concourse.mybir as mybir
from concourse import bacc, bass_utils
from concourse.masks import make_identity

BF16 = ml_dtypes.bfloat16
F32 = np.float32

B = 32
N = 577
C = 768
NH = 12
HD = 64
MLP = 3072
EPS = 1e-6
SCALE = HD ** (-0.5)

N_CORES = 8
BPC = B // N_CORES          # 4 batch items per core
NPAD = 640                  # per-item padded token count (5 * 128)
TOK = BPC * NPAD            # 2560 padded tokens per core
NCH = TOK // 128            # 20 token chunks
KC = C // 128               # 6 contraction chunks for dim 768
MC = MLP // 128             # 24 chunks for MLP dim
MCHUNK = NPAD // 128        # 5 m-chunks per batch item
VW = NH * 66 + 62           # v slab width (854): per head 64 v dims + 2 ones
F32T = mybir.dt.float32
BF16T = mybir.dt.bfloat16
AF = mybir.ActivationFunctionType
OP = mybir.AluOpType

SPLITS_N = [(0, 512), (512, 65)]   # 577-wide outputs (PSUM bank = 512 fp32)
SPLITS_C = [(0, 512), (512, 256)]  # 768-wide outputs (bank-aligned)

DEBUG_OUTPUTS = False  # expose xh/x2/xh2 scratch as outputs for bisection


def build_program(nc):
    dt = mybir.dt

    x_d = nc.dram_tensor("x", [TOK, C], dt.float32, kind="ExternalInput")
    xb_d = nc.dram_tensor("xb", [TOK, C], dt.bfloat16, kind="ExternalInput")
    wqk_d = nc.dram_tensor("wqkT", [C, 2 * C], dt.bfloat16, kind="ExternalInput")
    bqk_d = nc.dram_tensor("bias_qk", [2 * C], dt.float32, kind="ExternalInput")
    wv_d = nc.dram_tensor("wvT", [C, C], dt.bfloat16, kind="ExternalInput")
    bv_d = nc.dram_tensor("bias_v", [C], dt.bfloat16, kind="ExternalInput")
    wp_d = nc.dram_tensor("wprojT", [C, C], dt.bfloat16, kind="ExternalInput")
    w1_d = nc.dram_tensor("w1T", [C, MLP], dt.bfloat16, kind="ExternalInput")
    b1_d = nc.dram_tensor("bias_fc1", [MLP], dt.float32, kind="ExternalInput")
    w2_d = nc.dram_tensor("w2T", [MLP, C], dt.bfloat16, kind="ExternalInput")
    b2_d = nc.dram_tensor("bias_fc2", [C], dt.bfloat16, kind="ExternalInput")
    rpb_d = nc.dram_tensor("rpbT", [NH, 128, MCHUNK, N], dt.bfloat16,
                           kind="ExternalInput")
    out_d = nc.dram_tensor("out", [TOK, C], dt.float32, kind="ExternalOutput")

    skind = "ExternalOutput" if DEBUG_OUTPUTS else "Internal"
    xh_d = nc.dram_tensor("xh_scratch", [TOK, C], dt.bfloat16, kind=skind)
    xh2_d = nc.dram_tensor("xh2_scratch", [TOK, C], dt.bfloat16, kind=skind)
    x2_d = nc.dram_tensor("x2_scratch", [TOK, C], dt.bfloat16, kind=skind)
    rec_d = nc.dram_tensor("rec_scratch", [BPC, NH, N], dt.float32)

    x_ap = x_d.ap().rearrange("(c p) d -> p c d", p=128)      # [128, 20, 768]
    xb_ap = xb_d.ap().rearrange("(c p) d -> p c d", p=128)
    xh_ap = xh_d.ap().rearrange("(c p) d -> p c d", p=128)
    xh2_ap = xh2_d.ap().rearrange("(c p) d -> p c d", p=128)
    x2_ap = x2_d.ap().rearrange("(c p) d -> p c d", p=128)
    out_ap = out_d.ap().rearrange("(c p) d -> p c d", p=128)

    with tile.TileContext(nc) as tc, ExitStack() as ctx:
        persist = ctx.enter_context(tc.tile_pool(name="persist", bufs=1))
        psS = ctx.enter_context(tc.tile_pool(name="psS", bufs=2, space="PSUM"))
        psPV = ctx.enter_context(tc.tile_pool(name="psPV", bufs=2, space="PSUM"))

        eps_sb = persist.tile([128, 1], F32T, tag="eps")
        nc.vector.memset(eps_sb[:], EPS)
        ident = persist.tile([128, 128], BF16T, tag="ident")
        make_identity(nc, ident[:])
        bqk_sb = persist.tile([128, 12], F32T, tag="bqk")
        nc.sync.dma_start(bqk_sb[:], bqk_d.ap().rearrange("(m p) -> p m", p=128))
        bv_sb = persist.tile([128, C], BF16T, tag="bv")
        bvsrc = bv_d.ap()
        nc.sync.dma_start(bv_sb[:], bass.AP(
            tensor=bvsrc.tensor, offset=bvsrc.offset,
            ap=[[0, 128]] + list(bvsrc.ap)))
        bfc1_sb = persist.tile([128, MC], F32T, tag="bfc1")
        nc.sync.dma_start(bfc1_sb[:], b1_d.ap().rearrange("(m p) -> p m", p=128))
        bfc2_sb = persist.tile([128, C], BF16T, tag="bfc2")
        b2src = b2_d.ap()
        nc.sync.dma_start(bfc2_sb[:], bass.AP(
            tensor=b2src.tensor, offset=b2src.offset,
            ap=[[0, 128]] + list(b2src.ap)))
        mv20 = persist.tile([128, NCH, 2], F32T, tag="mv20")
        rstd20 = persist.tile([128, NCH], F32T, tag="rstd20")
        wqk_sb = persist.tile([128, KC, 2 * C], BF16T, tag="wqk")
        wv_sb = persist.tile([128, KC, C], BF16T, tag="wv")
        wp_sb = persist.tile([128, KC, C], BF16T, tag="wp")

        # qkv weights first (qkv(0) blocks on them), then x stats, then the
        # later-needed weights.
        nc.sync.dma_start(
            wqk_sb[:], wqk_d.ap().rearrange("(k p) c -> p k c", p=128))

        # ---------- prologue: LN1 statistics for all items ----------
        with tc.tile_pool(name="ln1stats", bufs=3) as lp:
            for b in range(BPC):
                for ic in range(MCHUNK):
                    i = b * MCHUNK + ic
                    sx = lp.tile([128, C], F32T, tag="sx")
                    nc.sync.dma_start(sx[:], x_ap[:, i, :])
                    st = lp.tile([128, 2, 6], F32T, tag="st")
                    nc.vector.bn_stats(st[:, 0, :], sx[:, 0:C // 2])
                    nc.vector.bn_stats(st[:, 1, :], sx[:, C // 2:C])
                    nc.vector.bn_aggr(mv20[:, i, :], st[:])
                sd = lp.tile([128, MCHUNK], F32T, tag="sd")
                nc.scalar.activation(
                    sd[:], mv20[:, b * MCHUNK:(b + 1) * MCHUNK, 1:2],
                    AF.Sqrt, bias=eps_sb[:, 0:1])
                nc.vector.reciprocal(
                    rstd20[:, b * MCHUNK:(b + 1) * MCHUNK], sd[:])

        nc.sync.dma_start(
            wv_sb[:], wv_d.ap().rearrange("(k p) c -> p k c", p=128))
        nc.sync.dma_start(wp_sb[:], wp_d.ap().rearrange("(k p) c -> p k c", p=128))

        # ---------- attention-superphase scope ----------
        abc_ctx = ExitStack()
        xhp = abc_ctx.enter_context(tc.tile_pool(name="xhp", bufs=1))
        qp = abc_ctx.enter_context(tc.tile_pool(name="qp", bufs=2))
        kp = abc_ctx.enter_context(tc.tile_pool(name="kp", bufs=3))
        vp = abc_ctx.enter_context(tc.tile_pool(name="vp", bufs=2))
        ep = abc_ctx.enter_context(tc.tile_pool(name="ep", bufs=3))
        ptp = abc_ctx.enter_context(tc.tile_pool(name="ptp", bufs=4))
        osbp = abc_ctx.enter_context(tc.tile_pool(name="osbp", bufs=14))
        smallp = abc_ctx.enter_context(tc.tile_pool(name="smallp", bufs=2))
        rbp = abc_ctx.enter_context(tc.tile_pool(name="rbp", bufs=2))
        lnx = abc_ctx.enter_context(tc.tile_pool(name="lnx", bufs=2))
        fin = abc_ctx.enter_context(tc.tile_pool(name="fin", bufs=2))

        def ln1_replay_steps(b):
            """Normalize item b's x chunks with the prologue stats -> xh.
            DMA loads are staggered one step ahead of the computes."""
            xts = {}

            def dma(ic):
                i = b * MCHUNK + ic
                xt = lnx.tile([128, C], F32T, tag="xt")
                nc.sync.dma_start(xt[:], x_ap[:, i, :])
                xts[ic] = xt

            def comp(ic):
                i = b * MCHUNK + ic
                xh_t = lnx.tile([128, C], BF16T, tag="xht")
                nc.vector.tensor_scalar(
                    out=xh_t[:], in0=xts.pop(ic)[:], scalar1=mv20[:, i, 0:1],
                    scalar2=rstd20[:, i:i + 1], op0=OP.subtract, op1=OP.mult)
                nc.sync.dma_start(xh_ap[:, i, :], xh_t[:])

            steps = [lambda: dma(0), lambda: dma(1), lambda: comp(0)]
            for ic in range(2, MCHUNK):
                steps.append(lambda ic=ic: dma(ic))
                steps.append(lambda ic=ic: comp(ic - 1))
            steps.append(lambda: comp(MCHUNK - 1))
            return steps

        def emit_transpose(b):
            xhT = xhp.tile([128, KC, NPAD], BF16T, tag="xhT")
            nc.sync.dma_start_transpose(
                xhT[:], xh_d.ap()[b * NPAD:(b + 1) * NPAD, :])
            return xhT

        def qkv_steps(b, xhT):
            """Closures computing q/k/v for item b into fresh per-b slabs.
            kT doubles as the (normalized) o^T slab for proj later."""
            cell = {}

            def alloc_step():
                qT = qp.tile([128, KC, NPAD], BF16T, tag="qT")
                kT = kp.tile([128, KC, NPAD], BF16T, tag="kT")
                v_sb = vp.tile([128, MCHUNK, VW], BF16T, tag="v")
                nc.vector.memset(
                    v_sb[:, :, 0:NH * 66].rearrange(
                        "p m (h e) -> p m h e", e=66)[:, :, :, 64:66], 1.0)
                nc.vector.memset(v_sb[:, :, NH * 66:], 0.0)
                nc.vector.memset(kT[:, :, N:NPAD], 0.0)
                cell['qT'], cell['kT'], cell['v'] = qT, kT, v_sb

            def qk_step(oc):
                ps = psS.tile([128, C], F32T, tag="s")
                for (lo, w) in SPLITS_N:
                    for kc in range(KC):
                        nc.tensor.matmul(
                            ps[:, lo:lo + w],
                            lhsT=wqk_sb[:, kc, oc * 128:(oc + 1) * 128],
                            rhs=xhT[:, kc, lo:lo + w],
                            start=(kc == 0), stop=(kc == KC - 1))
                dst = (cell['qT'][:, oc, 0:N] if oc < 6
                       else cell['kT'][:, oc - 6, 0:N])
                nc.vector.tensor_scalar(
                    out=dst, in0=ps[:, 0:N],
                    scalar1=bqk_sb[:, oc:oc + 1], scalar2=None,
                    op0=OP.add)

            def v_step(mc):
                mw = 128 if mc < MCHUNK - 1 else N - 4 * 128
                ps = psS.tile([128, C], F32T, tag="s")
                for (lo, w) in SPLITS_C:
                    for kc in range(KC):
                        nc.tensor.matmul(
                            ps[:mw, lo:lo + w],
                            lhsT=xhT[:, kc, mc * 128: mc * 128 + mw],
                            rhs=wv_sb[:, kc, lo:lo + w],
                            start=(kc == 0), stop=(kc == KC - 1))
                nc.vector.tensor_tensor(
                    cell['v'][0:mw, mc, 0:NH * 66].rearrange(
                        "p (h e) -> p h e", e=66)[:, :, 0:64],
                    ps[0:mw, 0:768].rearrange("p (h e) -> p h e", h=NH),
                    bv_sb[0:mw, :].rearrange("p (h e) -> p h e", h=NH),
                    OP.add)

            steps = [alloc_step]
            for oc in range(12):
                steps.append(lambda oc=oc: qk_step(oc))
            for mc in range(MCHUNK):
                steps.append(lambda mc=mc: v_step(mc))
            return steps, cell

        class PvPair:
            """Pending P^T @ [v|1] for a head pair, drained a few matmuls at
            a time during the next pair's S waves; den rows drain to den12
            (ACT + DMA row move), o rows drain to SBUF (DVE)."""

            def __init__(self, pts, v_sb, hp, den12, o_list):
                self.pts, self.v_sb, self.hp = pts, v_sb, hp
                self.den12, self.o_list = den12, o_list
                self.pv = [psPV.tile([128, C], F32T, tag="pv", name="pv")
                           for _ in range(2)]
                self.mms = []
                for h01 in range(2):
                    for (lo, w) in SPLITS_N:
                        for mc in range(MCHUNK):
                            self.mms.append((h01, mc, lo, w))
                self.pos = 0

            def drain(self, k):
                end = min(self.pos + k, len(self.mms))
                for (h01, mc, lo, w) in self.mms[self.pos:end]:
                    h = 2 * self.hp + h01
                    mw = 128 if mc < MCHUNK - 1 else N - 4 * 128
                    nc.tensor.matmul(
                        self.pv[h01][:, lo:lo + w],
                        lhsT=self.v_sb[0:mw, mc, h * 66: h * 66 + 128],
                        rhs=self.pts[h01][0:mw, mc, lo:lo + w],
                        start=(mc == 0), stop=(mc == MCHUNK - 1))
                self.pos = end
                if self.pos == len(self.mms):
                    for h01 in range(2):
                        h = 2 * self.hp + h01
                        pv = self.pv[h01]
                        dd = smallp.tile([1, N], F32T, tag="dd", bufs=4)
                        nc.scalar.activation(dd[:], pv[64:65, 0:N],
                                             AF.Identity, bias=0.0)
                        nc.sync.dma_start(self.den12[h:h + 1, :], dd[:])
                        o_sb = osbp.tile([64, N], BF16T, tag="osb")
                        nc.vector.tensor_scalar(
                            out=o_sb[:], in0=pv[0:64, 0:N], scalar1=1.0,
                            scalar2=None, op0=OP.mult)
                        self.o_list[h] = o_sb
                    self.pv = None
                    return True
                return False

            def finish(self):
                self.drain(len(self.mms))

        def emit_s_wave(qT, kT, hp, mc, pts, e_tiles):
            mw = 128 if mc < MCHUNK - 1 else N - 4 * 128
            ms = slice(mc * 128, mc * 128 + mw)
            sps = [psS.tile([128, C], F32T, tag="s", name="sps")
                   for _ in range(2)]
            for (lo, w) in SPLITS_N:
                for h01 in range(2):
                    pb = 64 * h01
                    nc.tensor.matmul(
                        sps[h01][:mw, lo:lo + w],
                        lhsT=kT[pb:pb + 64, hp, ms],
                        rhs=qT[pb:pb + 64, hp, lo:lo + w],
                        start=True, stop=False)
                for h01 in range(2):
                    nc.tensor.matmul(
                        sps[h01][:mw, lo:lo + w],
                        lhsT=ident[0:mw, 0:mw],
                        rhs=e_tiles[h01][0:mw, mc, lo:lo + w],
                        start=False, stop=True)
            for h01 in range(2):
                nc.scalar.activation(pts[h01][:mw, mc, 0:N],
                                     sps[h01][:mw, 0:N], AF.Exp)

        def finalize_steps(b, kT, den12, o_list):
            """o normalize into kT slab + proj + residual + LN2 (batched
            sqrt) for item b."""
            mv10 = fin.tile([128, MCHUNK, 2], F32T, tag="mv10")
            rstd5 = fin.tile([128, MCHUNK], F32T, tag="rstd5")
            x2ts = {}
            steps = []

            def recip_step():
                rec12 = smallp.tile([12, N], F32T, tag="rec")
                nc.vector.reciprocal(rec12[:], den12[:])
                nc.sync.dma_start(rec_d.ap()[b], rec12[:])
            steps.append(recip_step)

            def mult_step(h):
                base = 64 * (h % 2)
                rb = rbp.tile([64, N], F32T, tag="rb")
                rsrc = rec_d.ap()[b, h]
                nc.sync.dma_start(rb[:], bass.AP(
                    tensor=rsrc.tensor, offset=rsrc.offset,
                    ap=[[0, 64]] + list(rsrc.ap)))
                nc.vector.tensor_tensor(
                    kT[base:base + 64, h // 2, 0:N], o_list[h][:],
                    rb[:], OP.mult)
            for h in range(NH):
                steps.append(lambda h=h: mult_step(h))

            def proj_chunk(ic):
                i = b * MCHUNK + ic
                xbt = fin.tile([128, C], BF16T, tag="xbt")
                nc.sync.dma_start(xbt[:], xb_ap[:, i, :])
                ps = psS.tile([128, C], F32T, tag="s")
                for (lo, w) in SPLITS_C:
                    for cc in range(KC):
                        nc.tensor.matmul(
                            ps[:, lo:lo + w],
                            lhsT=kT[:, cc, ic * 128:(ic + 1) * 128],
                            rhs=wp_sb[:, cc, lo:lo + w],
                            start=(cc == 0), stop=(cc == KC - 1))
                x2t = fin.tile([128, C], BF16T, tag="x2t", bufs=6)
                nc.vector.tensor_tensor(x2t[:], ps[:, 0:C], xbt[:], OP.add)
                x2ts[ic] = x2t
                nc.sync.dma_start(x2_ap[:, i, :], x2t[:])
                st = fin.tile([128, 2, 6], F32T, tag="st2")
                nc.vector.bn_stats(st[:, 0, :], x2t[:, 0:C // 2])
                nc.vector.bn_stats(st[:, 1, :], x2t[:, C // 2:C])
                nc.vector.bn_aggr(mv10[:, ic, :], st[:])
            for ic in range(MCHUNK):
                steps.append(lambda ic=ic: proj_chunk(ic))

            def rsqrt_step():
                sd5 = fin.tile([128, MCHUNK], F32T, tag="sd5")
                nc.scalar.activation(sd5[:], mv10[:, :, 1:2], AF.Sqrt,
                                     bias=eps_sb[:, 0:1])
                nc.vector.reciprocal(rstd5[:], sd5[:])
            steps.append(rsqrt_step)

            def norm_step(ic):
                i = b * MCHUNK + ic
                xh2t = fin.tile([128, C], BF16T, tag="xh2t")
                nc.vector.tensor_scalar(
                    out=xh2t[:], in0=x2ts.pop(ic)[:],
                    scalar1=mv10[:, ic, 0:1], scalar2=rstd5[:, ic:ic + 1],
                    op0=OP.subtract, op1=OP.mult)
                nc.sync.dma_start(xh2_ap[:, i, :], xh2t[:])
            for ic in range(MCHUNK):
                steps.append(lambda ic=ic: norm_step(ic))
            return steps

        def load_e_pair(hp):
            tiles = []
            for h in (2 * hp, 2 * hp + 1):
                et = ep.tile([128, MCHUNK, N], BF16T, tag="E")
                nc.sync.dma_start(et[:], rpb_d.ap()[h])
                tiles.append(et)
            return tiles

        # ---------------- main pipelined loop ----------------
        for s in ln1_replay_steps(0):
            s()
        xhT_cur = emit_transpose(0)
        q_steps, q_cell = qkv_steps(0, xhT_cur)
        for s in q_steps:
            s()
        cur = q_cell
        pending = None
        states = {}
        e_next = load_e_pair(0)
        for b in range(BPC):
            qT, kT, v_sb = cur['qT'], cur['kT'], cur['v']
            den12 = smallp.tile([12, N], F32T, tag="den")
            o_list = [None] * NH
            states[b] = (den12, o_list)
            carry = pending  # last pair of item b-1; gates finalize(b-1)
            sec = []
            if b + 1 < BPC:
                sec += ln1_replay_steps(b + 1)
                q_next = {}

                def transpose_and_plan(b=b):
                    xhT = emit_transpose(b + 1)
                    steps, cellv = qkv_steps(b + 1, xhT)
                    q_next['cell'] = cellv
                    sec.extend(steps)
                sec.append(transpose_and_plan)
            sec_fin = []
            if b > 0:
                sec_fin = finalize_steps(b - 1, prev_kT, *states.pop(b - 1))
            sec_i = 0
            fin_i = 0

            def pump(k):
                nonlocal sec_i, fin_i
                run = 0
                while run < k:
                    if sec_i < len(sec):
                        sec[sec_i]()
                        sec_i += 1
                    elif carry is None and fin_i < len(sec_fin):
                        sec_fin[fin_i]()
                        fin_i += 1
                    else:
                        return
                    run += 1

            for hp in range(6):
                e_tiles = e_next
                if hp < 5:
                    e_next = load_e_pair(hp + 1)
                elif b + 1 < BPC:
                    e_next = load_e_pair(0)
                pts = [ptp.tile([128, MCHUNK, N], BF16T, tag="pt", name="pt")
                       for _ in range(2)]
                for mc in range(MCHUNK):
                    emit_s_wave(qT, kT, hp, mc, pts, e_tiles)
                    if pending is not None:
                        if pending.drain(4 if mc < MCHUNK - 1 else 20):
                            if pending is carry:
                                carry = None
                            pending = None
                    pump(2)
                pending = PvPair(pts, v_sb, hp, den12, o_list)
            # drain remaining secondary steps for this b
            pump(10 ** 9)
            assert sec_i == len(sec) and fin_i == len(sec_fin)
            prev_kT = kT
            if b + 1 < BPC:
                cur = q_next['cell']
        if pending is not None:
            pending.finish()
            pending = None
        for s in finalize_steps(BPC - 1, prev_kT, *states.pop(BPC - 1)):
            s()

        abc_ctx.close()

        # ================= MLP (bf16) =================
        with ExitStack() as mctx:
            mlpw = mctx.enter_context(tc.tile_pool(name="mlpw", bufs=1))
            w1_sb = mlpw.tile([128, KC, MLP], BF16T, tag="w1")
            w1_src = w1_d.ap().rearrange("(k p) c -> p k c", p=128)
            for kc in range(KC):
                nc.sync.dma_start(w1_sb[:, kc, :], w1_src[:, kc, :])
            w2_sb = mlpw.tile([128, MC, C], BF16T, tag="w2")
            w2_src = w2_d.ap().rearrange("(k p) c -> p k c", p=128)
            for mc8 in range(4):
                nc.sync.dma_start(w2_sb[:, mc8 * 6:(mc8 + 1) * 6, :],
                                  w2_src[:, mc8 * 6:(mc8 + 1) * 6, :])
            mtp = mctx.enter_context(tc.tile_pool(name="mtp", bufs=1))
            ck3 = mctx.enter_context(tc.tile_pool(name="mlpck", bufs=2))
            NB = 512
            for nb in range(TOK // NB):
                xh2T = ck3.tile([128, KC, NB], BF16T, tag="xh2T")
                nc.sync.dma_start_transpose(
                    xh2T[:], xh2_d.ap()[nb * NB:(nb + 1) * NB, :])
                mt = mtp.tile([128, MC, NB], BF16T, tag="mt")
                for mc in range(MC):
                    mps = psS.tile([128, C], F32T, tag="s")
                    for kc in range(KC):
                        nc.tensor.matmul(
                            mps[:, 0:NB],
                            lhsT=w1_sb[:, kc, mc * 128:(mc + 1) * 128],
                            rhs=xh2T[:, kc, :],
                            start=(kc == 0), stop=(kc == KC - 1))
                    nc.scalar.activation(mt[:, mc, :], mps[:, 0:NB], AF.Gelu,
                                         bias=bfc1_sb[:, mc:mc + 1])
                for ns in range(NB // 128):
                    i = nb * (NB // 128) + ns
                    xf = ck3.tile([128, C], BF16T, tag="xf")
                    nc.sync.dma_start(xf[:], x2_ap[:, i, :])
                    fps = psPV.tile([128, C], F32T, tag="pv")
                    for (lo, w) in SPLITS_C:
                        for mc in range(MC):
                            nc.tensor.matmul(
                                fps[:, lo:lo + w],
                                lhsT=mt[:, mc, ns * 128:(ns + 1) * 128],
                                rhs=w2_sb[:, mc, lo:lo + w],
                                start=(mc == 0), stop=(mc == MC - 1))
                    ot = ck3.tile([128, C], F32T, tag="ot")
                    nc.vector.tensor_tensor(ot[:], fps[:, 0:C], xf[:], OP.add)
                    nc.vector.tensor_tensor(ot[:], ot[:], bfc2_sb[:], OP.add)
                    nc.sync.dma_start(out_ap[:, i, :], ot[:])


def host_prep(inputs):
    """Fold layernorms/biases/scale into weights; build per-core input maps."""
    x = np.asarray(inputs['x'], F32)
    qkv_w = np.asarray(inputs['qkv_w'], F32)
    g1 = np.asarray(inputs['norm1_g'], F32)
    b1 = np.asarray(inputs['norm1_b'], F32)
    q_bias = np.asarray(inputs['q_bias'], F32)
    v_bias = np.asarray(inputs['v_bias'], F32)
    rpb_table = np.asarray(inputs['rpb_table'], F32)
    rel_index = np.asarray(inputs['rel_index'])
    proj_w = np.asarray(inputs['proj_w'], F32)
    proj_b = np.asarray(inputs['proj_b'], F32)
    g2 = np.asarray(inputs['norm2_g'], F32)
    b2 = np.asarray(inputs['norm2_b'], F32)
    fc1_w = np.asarray(inputs['fc1_w'], F32)
    fc1_b = np.asarray(inputs['fc1_b'], F32)
    fc2_w = np.asarray(inputs['fc2_w'], F32)
    fc2_b = np.asarray(inputs['fc2_b'], F32)

    Wq = qkv_w[0:C] * g1[None, :] * SCALE
    bias_q = (qkv_w[0:C] @ b1 + q_bias) * SCALE
    Wk = qkv_w[C:2 * C] * g1[None, :]
    bias_k = qkv_w[C:2 * C] @ b1
    Wv = qkv_w[2 * C:] * g1[None, :]
    bias_v = qkv_w[2 * C:] @ b1 + v_bias

    wqkT = np.ascontiguousarray(np.concatenate([Wq, Wk], 0).T).astype(BF16)
    bias_qk = np.concatenate([bias_q, bias_k]).astype(F32)
    wvT = np.ascontiguousarray(Wv.T).astype(BF16)
    wprojT = np.ascontiguousarray(proj_w.T).astype(BF16)
    w1T = np.ascontiguousarray((fc1_w * g2[None, :]).T).astype(BF16)
    bias_fc1 = (fc1_w @ b2 + fc1_b).astype(F32)
    w2T = np.ascontiguousarray(fc2_w.T).astype(BF16)

    rpb = rpb_table[rel_index]                     # [N, N, NH]
    # rpbT[h, p, mc, n] = rpb[n, m, h] with m = mc*128 + p
    rt = rpb.transpose(2, 1, 0)                    # [NH, m, n]
    rpad = np.zeros((NH, NPAD, N), F32)
    rpad[:, :N, :] = rt
    rpbT = np.ascontiguousarray(
        rpad.reshape(NH, MCHUNK, 128, N).transpose(0, 2, 1, 3)).astype(BF16)

    shared = dict(
        wqkT=wqkT, bias_qk=bias_qk, wvT=wvT, bias_v=bias_v.astype(BF16),
        wprojT=wprojT, w1T=w1T, bias_fc1=bias_fc1, w2T=w2T,
        bias_fc2=fc2_b.astype(BF16), rpbT=rpbT)

    xpad = np.zeros((B, NPAD, C), F32)
    xpad[:, :N, :] = x
    xbpad = (xpad + proj_b[None, None, :].astype(F32)).astype(BF16)
    in_maps = []
    for core in range(N_CORES):
        xi = xpad[core * BPC:(core + 1) * BPC].reshape(TOK, C)
        xbi = xbpad[core * BPC:(core + 1) * BPC].reshape(TOK, C)
        m = dict(shared)
        m['x'] = np.ascontiguousarray(xi)
        m['xb'] = np.ascontiguousarray(xbi)
        in_maps.append(m)
    return in_maps


def build_bass():
    nc = bacc.Bacc("TRN2", target_bir_lowering=False, debug=False,
                   num_devices=N_CORES)
    build_program(nc)
    nc.compile()
    return nc


def gather_output(results):
    out = np.zeros((B, N, C), F32)
    for core in range(N_CORES):
        o = results[core]["out"].reshape(BPC, NPAD, C)
        out[core * BPC:(core + 1) * BPC] = o[:, :N, :]
    return out


def kernel(**inputs):
    in_maps = host_prep(inputs)
    nc = build_bass()
    res = bass_utils.run_bass_kernel_spmd(nc, in_maps,
                                          core_ids=list(range(N_CORES)))
    return gather_output(res.results)
